# revision 1
# baseline (speedup 1.0000x reference)
"""Multi-head attention (B=4, N=2048, C=256, H=8) on 8 Trainium2 NeuronCores.

Sharding: core c handles batch b = c//2 and query-half qh = c%2 (1024 query
rows), all 8 heads. k/v are computed for the full sequence on each core (the
qkv projection is cheap); outputs concatenate with no cross-core reduction.

Device-side layout is fully "transposed" (channels on partitions):
  - x^T [C, N] feeds q^T/k^T ([d, tokens], head-major rows) and v ([tokens, d]).
  - Scores are computed as S^T [k-tokens, q-tokens] so that softmax's exp
    output E^T feeds the AV matmul directly (contraction over k on partitions).
  - Softmax denominators come for free as a 33rd "ones" column appended to v.
  - O^T [channels, q] feeds the output projection directly.
Softmax skips max-subtraction: scaled scores are ~N(0,1) (max |s| < ~10), safe
in fp32 exp. Matmuls run in float32r (full PE rate at free dim >= 256); every
float32r operand is produced by a rounding-capable instruction (DVE copy, ACT).

The emission is software-pipelined around the ScalarE (exp) bottleneck:
  - AV matmuls trail their chunk by one tick so the PE never sits between
    consecutive ACT ops at iteration boundaries.
  - Most of the qkv projection is spread across the first iterations' chunk
    loop (worklist), so exp starts after a minimal prefix.
  - Normalization and the output projection of a finished q-block are emitted
    a few chunks into the following iteration.
"""

import os
from contextlib import ExitStack

import numpy as np

import concourse.bacc as bacc
import concourse.bass as bass
import concourse.mybir as mybir
import concourse.tile as tile
from concourse.bass_utils import run_bass_kernel_spmd

B, N, C = 4, 2048, 256
H, D = 8, 32
P = 128
QH = N // 2              # query rows per core
SCALE = float(D) ** -0.5
NCORES = 8
NCH = N // P             # 16 k-chunks

F32 = mybir.dt.float32
F32R = mybir.dt.float32r
EXP = mybir.ActivationFunctionType.Exp
LN = mybir.ActivationFunctionType.Ln

# Timing amplification for the local harness (repeat attention+proj body).
# REPS read per-call via _reps_key() so one process can time several variants.
# Timing-decomposition variants (break correctness, timing only):
#   "" (normal), "noact" (constant E feed), "justs" (S^T only, no exp/AV),
#   "actsb" (stage scores psum->sbuf on DVE; exp reads SBUF)
VARIANT = os.environ.get("BASS_ATTN_VARIANT", "")


def _emit(tc, xT, xTq, wall, pb, y, REPS, REPS_MODE):
    nc = tc.nc
    with ExitStack() as ctx:
        singles = ctx.enter_context(tc.tile_pool(name="singles", bufs=1))
        epool = ctx.enter_context(
            tc.tile_pool(name="epool", bufs=2 if VARIANT == "actsb" else 3)
        )
        odp = ctx.enter_context(tc.tile_pool(name="odp", bufs=3))
        small = ctx.enter_context(tc.tile_pool(name="small", bufs=4))
        ypool = ctx.enter_context(tc.tile_pool(name="ypool", bufs=3))
        # PSUM budget (8 banks): s-tiles 2x2 + po 3x1 + bcp 1x1
        ps = ctx.enter_context(tc.tile_pool(name="ps", bufs=2, space="PSUM"))
        po = ctx.enter_context(tc.tile_pool(name="po", bufs=3, space="PSUM"))
        bcp = ctx.enter_context(tc.tile_pool(name="bcp", bufs=1, space="PSUM"))

        # ---- input loads + fp32r rounding (staged) -----------------------
        def load_r(name, dram_ap, cshape):
            ld = singles.tile(cshape, F32, tag=name + "_ld", name=name + "_ld")
            nc.sync.dma_start(ld[:], dram_ap)
            rt = singles.tile(cshape, F32R, tag=name, name=name)
            nc.vector.tensor_copy(rt[:], ld[:])
            return rt

        # all four weight matrices arrive packed in one dram tensor (one DMA,
        # one rounding copy); order: wk, wq, wv, pw
        wall_ld = singles.tile([P, 2, 4 * C], F32, tag="wall_ld", name="wall_ld")
        wall_sb = singles.tile([P, 2, 4 * C], F32R, tag="wall", name="wall_sb")
        wall4 = wall_sb[:].rearrange("p c (w n) -> p c w n", n=C)
        wk_sb = wall4[:, :, 0]
        wq_sb = wall4[:, :, 1]
        wv_sb = wall4[:, :, 2]
        pw_sb = wall4[:, :, 3]
        # x^T loads staged + rounded in 512-column blocks so the first qkv
        # matmuls (and the attention stream behind them) start early.
        xT_ld = singles.tile([P, 2, N], F32, tag="xT_ld", name="xT_ld")
        xT_sb = singles.tile([P, 2, N], F32R, tag="xT", name="xT_sb")
        xT_r = xT.rearrange("(c p) n -> p c n", p=P)

        def load_x_block(nb):
            sl = (slice(None), slice(None), slice(512 * nb, 512 * nb + 512))
            nc.sync.dma_start(xT_ld[sl], xT_r[sl])
            nc.vector.tensor_copy(xT_sb[sl], xT_ld[sl])

        load_x_block(0)
        nc.sync.dma_start(wall_ld[:], wall.rearrange("(c p) n -> p c n", p=P))
        # round the k-projection slice first: it gates the first matmuls
        nc.vector.tensor_copy(wall_sb[:, :, 0:C], wall_ld[:, :, 0:C])
        nc.vector.tensor_copy(wall_sb[:, :, C : 4 * C], wall_ld[:, :, C : 4 * C])
        xTq_ld = singles.tile([P, 2, QH], F32, tag="xTq_ld", name="xTq_ld")
        xTq_sb = singles.tile([P, 2, QH], F32R, tag="xTq", name="xTq_sb")
        xTq_r = xTq.rearrange("(c p) n -> p c n", p=P)

        def load_xq_block(nb):
            sl = (slice(None), slice(None), slice(512 * nb, 512 * nb + 512))
            nc.sync.dma_start(xTq_ld[sl], xTq_r[sl])
            nc.vector.tensor_copy(xTq_sb[sl], xTq_ld[sl])

        load_xq_block(0)
        for nb in range(1, 4):
            load_x_block(nb)
        load_xq_block(1)
        pb_sb = singles.tile([P, C], F32, tag="pb")
        nc.sync.dma_start(
            pb_sb[:],
            bass.AP(tensor=pb.tensor, offset=pb.offset, ap=[[0, P]] + list(pb.ap)),
        )

        # ---- qkv projection emitters ------------------------------------
        # q^T/k^T stacks: chunk cc holds heads 4cc..4cc+3 at rows 32*(h%4).
        qT_sb = singles.tile([P, 2, QH], F32R, tag="qT")
        kT_sb = singles.tile([P, 2, N], F32R, tag="kT")
        # v_aug: [token-tile, head-major (v_h | 1)] for AV + denominator.
        vA_sb = singles.tile([P, NCH, H * (D + 1)], F32R, tag="vA")
        onesF = singles.tile([P, NCH, H], F32, tag="onesF")
        nc.vector.memset(onesF[:], 1.0)
        vA4 = vA_sb[:].rearrange("p t (h a) -> p t h a", a=D + 1)
        nc.vector.tensor_copy(vA4[:, :, :, D], onesF[:])

        def emit_kqT(w_sb, x_sb, out_sb, cc, nb):
            pk = bcp.tile([P, 512], F32, tag="bc", name="pk")
            for ci in range(2):
                nc.tensor.matmul(
                    pk[:],
                    lhsT=w_sb[:, ci, 128 * cc : 128 * cc + 128],
                    rhs=x_sb[:, ci, 512 * nb : 512 * nb + 512],
                    start=(ci == 0),
                    stop=(ci == 1),
                )
            nc.vector.tensor_copy(out_sb[:, cc, 512 * nb : 512 * nb + 512], pk[:])

        def emit_v(tt):
            pv = bcp.tile([P, 512], F32, tag="bc", name="pv")
            for ci in range(2):
                nc.tensor.matmul(
                    pv[:, 0:256],
                    lhsT=xT_sb[:, ci, 128 * tt : 128 * tt + 128],
                    rhs=wv_sb[:, ci, :],
                    start=(ci == 0),
                    stop=(ci == 1),
                )
            nc.vector.tensor_copy(
                vA4[:, tt, :, 0:D],
                pv[:, 0:256].rearrange("p (h d) -> p h d", d=D),
            )

        # ---- attention helpers ------------------------------------------
        ones_f2 = singles.tile([1, 32], F32, tag="onesf2")
        nc.vector.memset(ones_f2[:], 1.0)
        ones_sb = singles.tile([1, 32], F32R, tag="ones")
        nc.vector.tensor_copy(ones_sb[:], ones_f2[:])
        et_const = None
        if VARIANT in ("noact", "justs"):
            etc_f = singles.tile([P, 1024], F32, tag="etcf", name="etc_f")
            nc.vector.memset(etc_f[:], 0.5)
            et_const = singles.tile([P, 1024], F32R, tag="etc", name="et_const")
            nc.vector.tensor_copy(et_const[:], etc_f[:])
        OT_sb = singles.tile([P, 2, QH], F32R, tag="OT")

        def emit_proj_qt(qb, qt):
            tq = 4 * qb + qt
            py = bcp.tile([P, 512], F32, tag="bc", name="py")
            for ci in range(2):
                nc.tensor.matmul(
                    py[:, 0:256],
                    lhsT=OT_sb[:, ci, 128 * tq : 128 * tq + 128],
                    rhs=pw_sb[:, ci, :],
                    start=(ci == 0),
                    stop=(ci == 1),
                )
            ysb = ypool.tile([P, C], F32, tag="y", name="ysb")
            nc.vector.tensor_add(ysb[:], py[:, 0:256], pb_sb[:])
            nc.sync.dma_start(y[128 * tq : 128 * tq + 128, :], ysb[:])

        def emit_norm_head(pot, hp, qb, e):
            # normalize O^T of head 2hp+e (pot rows 0:32, denominator row 32)
            h = 2 * hp + e
            r, cc = 32 * (h % 4), h // 4
            rcpf = small.tile([1, 512], F32, tag="rcpf", name="rcpf")
            nc.vector.reciprocal(rcpf[:], pot[D : D + 1, :])
            rcp = small.tile([1, 512], F32R, tag="rcp", name="rcp")
            nc.vector.tensor_copy(rcp[:], rcpf[:])
            bc = bcp.tile([32, 512], F32, tag="bc", name="bc")
            nc.tensor.matmul(  # broadcast 1/den to 32 rows
                bc[:], lhsT=ones_sb[:], rhs=rcp[:], start=True, stop=True
            )
            onr = small.tile([32, 512], F32, tag="onr", name="onr")
            nc.vector.tensor_copy(onr[:], pot[0:D, :])
            nc.vector.tensor_mul(
                OT_sb[r : r + 32, cc, 512 * qb : 512 * qb + 512],
                onr[:],
                bc[:],
            )

        def emit_av(pots, hp, et, ch):
            # et: [P, 1024] AP (heads side by side) for chunk ch
            for e in range(2):
                h = 2 * hp + e
                nc.tensor.matmul(
                    pots[e][:],
                    lhsT=vA_sb[:, ch, (D + 1) * h : (D + 1) * (h + 1)],
                    rhs=et[:, 512 * e : 512 * e + 512],
                    start=(ch == 0),
                    stop=(ch == NCH - 1),
                    skip_group_check=True,
                )

        def emit_body():
            # ---- prefix: just enough qkv for iteration 0's first chunks ------
            emit_kqT(wk_sb, xT_sb, kT_sb, 0, 0)
            emit_kqT(wq_sb, xTq_sb, qT_sb, 0, 0)
            for tt in range(4):
                emit_v(tt)
            # remaining qkv work, spread one item per chunk tick; deadlines:
            # v_tt by tick tt, kT(0,nb) before tick 4nb, cc=1 before tick 32.
            worklist = [
                lambda: emit_v(4),
                lambda: emit_kqT(wk_sb, xT_sb, kT_sb, 0, 1),
                lambda: emit_v(5),
                lambda: emit_v(6),
                lambda: emit_v(7),
                lambda: emit_kqT(wk_sb, xT_sb, kT_sb, 0, 2),
                lambda: emit_v(8),
                lambda: emit_v(9),
                lambda: emit_v(10),
                lambda: emit_kqT(wk_sb, xT_sb, kT_sb, 0, 3),
                lambda: emit_v(11),
                lambda: emit_v(12),
                lambda: emit_v(13),
                lambda: emit_v(14),
                lambda: emit_v(15),
                lambda: emit_kqT(wq_sb, xTq_sb, qT_sb, 0, 1),
                lambda: emit_kqT(wk_sb, xT_sb, kT_sb, 1, 0),
                lambda: emit_kqT(wk_sb, xT_sb, kT_sb, 1, 1),
                lambda: emit_kqT(wk_sb, xT_sb, kT_sb, 1, 2),
                lambda: emit_kqT(wk_sb, xT_sb, kT_sb, 1, 3),
                lambda: emit_kqT(wq_sb, xTq_sb, qT_sb, 1, 0),
                lambda: emit_kqT(wq_sb, xTq_sb, qT_sb, 1, 1),
            ]

            # ---- attention main loop -----------------------------------------
            pend_av = None      # (pot, hp, et, ch): AV trails by one tick
            deferred = []       # norm/proj actions, one per tick
            body_reps = REPS if (REPS > 1 and REPS_MODE == "unroll") else 1
            its = [
                (qb, hp)
                for _ in range(body_reps)
                for qb in range(QH // 512)
                for hp in range(H // 2)
            ]
            for qb, hp in its:
                pots = None
                for ch in range(NCH):
                    s = ps.tile([P, 1024], F32, tag="s", name="s")
                    for e in range(2):
                        h = 2 * hp + e
                        r, cc = 32 * (h % 4), h // 4
                        nc.tensor.matmul(
                            s[:, 512 * e : 512 * e + 512],
                            lhsT=kT_sb[r : r + 32, cc, 128 * ch : 128 * ch + 128],
                            rhs=qT_sb[r : r + 32, cc, 512 * qb : 512 * qb + 512],
                            start=True,
                            stop=True,
                            tile_position=(r, 0),
                        )
                    if VARIANT == "noact":
                        et = et_const
                    elif VARIANT == "justs":
                        # keep S^T alive with a tiny DVE read; no exp, no AV
                        et = et_const
                        nc.vector.tensor_copy(
                            OT_sb[0:1, 0, 32 * ch : 32 * ch + 32].bitcast(F32),
                            s[0:1, 0:32],
                        )
                    elif VARIANT == "actsb":
                        # old design (baseline A/B): stage scores to SBUF on
                        # DVE, batched exp over 4 chunks reads SBUF
                        if ch % 4 == 0:
                            scw = epool.tile([P, 4, 1024], F32, tag="SC", name="scw")
                        nc.vector.tensor_copy(scw[:, ch % 4, :], s[:])
                        if ch % 4 == 3:
                            etw4 = epool.tile([P, 4, 1024], F32R, tag="E4", name="etw4")
                            nc.scalar.activation(etw4[:], scw[:], EXP, scale=SCALE)
                            for j in range(4):
                                emit_av(pots, hp, etw4[:, j, :], ch - 3 + j)
                    else:
                        # exp reads scores directly from PSUM (ScalarE is
                        # closer to PSUM; avoids a DVE staging copy, which
                        # made DVE the bottleneck engine at ~211us busy)
                        etw = epool.tile([P, 1024], F32R, tag="E", name="etw")
                        nc.scalar.activation(etw[:], s[:], EXP, scale=SCALE)
                        et = etw[:]
                    if pots is None and VARIANT != "justs":
                        pots = (
                            po.tile([D + 1, 512], F32, tag="o", name="pot0"),
                            po.tile([D + 1, 512], F32, tag="o", name="pot1"),
                        )
                    if VARIANT not in ("justs", "actsb"):
                        if pend_av is not None:
                            emit_av(*pend_av)
                        pend_av = (pots, hp, et, ch)
                    if deferred:
                        deferred.pop(0)()
                    elif worklist:
                        worklist.pop(0)()
                if VARIANT in ("justs", "nonorm"):
                    continue
                if (qb, hp) != its[-1]:
                    deferred.append(
                        lambda a=pots[0], b=hp, c=qb: emit_norm_head(a, b, c, 0)
                    )
                    deferred.append(
                        lambda a=pots[1], b=hp, c=qb: emit_norm_head(a, b, c, 1)
                    )
                    if hp == H // 2 - 1:
                        for qt in range(4):
                            deferred.append(lambda a=qb, b=qt: emit_proj_qt(a, b))
            if VARIANT in ("justs", "nonorm"):
                if VARIANT == "nonorm" and pend_av is not None:
                    emit_av(*pend_av)
                for tq in range(8):
                    ysb = ypool.tile([P, C], F32, tag="y", name="ysbj")
                    nc.vector.memset(ysb[:], 0.0)
                    nc.sync.dma_start(y[128 * tq : 128 * tq + 128, :], ysb[:])
                return
            if pend_av is not None:
                emit_av(*pend_av)
            for act in deferred:
                act()
            # fast epilogue for the final head pair: interleave both heads'
            # chains and broadcast with plain-fp32 matmuls (PE is idle here),
            # skipping the fp32r rounding copy on the reciprocal.
            l_qb, l_hp = its[-1]
            rcpfs = []
            for e in range(2):
                rcpf = small.tile([1, 512], F32, tag="rcpf", name="rcpf")
                nc.vector.reciprocal(rcpf[:], pots[e][D : D + 1, :])
                rcpfs.append(rcpf)
            for e in range(2):
                bc = ps.tile([P, 1024], F32, tag="s", name="bcT")
                nc.tensor.matmul(
                    bc[0:32, 0:512], lhsT=ones_f2[:], rhs=rcpfs[e][:],
                    start=True, stop=True,
                )
                onr = small.tile([32, 512], F32, tag="onr", name="onr")
                nc.vector.tensor_copy(onr[:], pots[e][0:D, :])
                h = 2 * l_hp + e
                r, cc = 32 * (h % 4), h // 4
                nc.vector.tensor_mul(
                    OT_sb[r : r + 32, cc, 512 * l_qb : 512 * l_qb + 512],
                    onr[:],
                    bc[0:32, 0:512],
                )
            for qt in range(4):
                tq = 4 * l_qb + qt
                py = ps.tile([P, 1024], F32, tag="s", name="pyT")
                for ci in range(2):
                    nc.tensor.matmul(
                        py[:, 0:256],
                        lhsT=OT_sb[:, ci, 128 * tq : 128 * tq + 128],
                        rhs=pw_sb[:, ci, :],
                        start=(ci == 0),
                        stop=(ci == 1),
                    )
                ysb = ypool.tile([P, C], F32, tag="y", name="ysb")
                nc.vector.tensor_add(ysb[:], py[:, 0:256], pb_sb[:])
                nc.sync.dma_start(y[128 * tq : 128 * tq + 128, :], ysb[:])

        if REPS == 1 or REPS_MODE == "unroll":
            emit_body()
        else:
            with tc.For_i(0, REPS, 1):
                emit_body()


_NC = {}
_RUNNER = {}


def _reps_key():
    reps = int(os.environ.get("BASS_ATTN_REPS", "1"))
    mode = os.environ.get("BASS_ATTN_REPS_MODE", "loop")
    return (reps, mode if reps > 1 else "loop")


def _get_runner():
    """Cached SPMD runner: builds the jitted shard_map executable once so warm
    kernel() calls skip JAX retracing/compilation (run_bass_kernel_spmd builds
    a fresh closure per call, which always misses the jit cache)."""
    key = _reps_key()
    if key in _RUNNER:
        return _RUNNER[key]
    import jax
    from jax.sharding import Mesh, PartitionSpec
    from jax.experimental.shard_map import shard_map
    from concourse import bass2jax, mybir as _mb

    nc = _get_nc()
    bass2jax.install_neuronx_cc_hook()

    assert nc.dbg_addr is None
    partition_name = nc.partition_id_tensor.name if nc.partition_id_tensor else None
    in_names, out_names, out_avals = [], [], []
    for alloc in nc.m.functions[0].allocations:
        if not isinstance(alloc, _mb.MemoryLocationSet):
            continue
        name = alloc.memorylocations[0].name
        if alloc.kind == "ExternalInput":
            if name != partition_name:
                in_names.append(name)
        elif alloc.kind == "ExternalOutput":
            out_names.append(name)
            out_avals.append(
                jax.core.ShapedArray(tuple(alloc.tensor_shape), _mb.dt.np(alloc.dtype))
            )
    n_params = len(in_names)
    n_outs = len(out_avals)
    all_names = in_names + out_names
    if partition_name is not None:
        all_names = all_names + [partition_name]

    def _body(*args):
        operands = list(args)
        if partition_name is not None:
            operands.append(bass2jax.partition_id_tensor())
        outs = bass2jax._bass_exec_p.bind(
            *operands,
            out_avals=tuple(out_avals),
            in_names=tuple(all_names),
            out_names=tuple(out_names),
            lowering_input_output_aliases=(),
            sim_require_finite=True,
            sim_require_nnan=True,
            nc=nc,
        )
        return tuple(outs)

    devices = jax.devices()[:NCORES]
    mesh = Mesh(np.asarray(devices), ("core",))
    sharded = jax.jit(
        shard_map(
            _body,
            mesh=mesh,
            in_specs=(PartitionSpec("core"),) * (n_params + n_outs),
            out_specs=(PartitionSpec("core"),) * n_outs,
            check_rep=False,
        ),
        donate_argnums=tuple(range(n_params, n_params + n_outs)),
        keep_unused=True,
    )

    def run(in_maps):
        concat_in = [
            np.concatenate([np.asarray(m[nm]) for m in in_maps], axis=0)
            for nm in in_names
        ]
        concat_zeros = [
            np.zeros((NCORES * a.shape[0], *a.shape[1:]), a.dtype) for a in out_avals
        ]
        out_arrs = sharded(*concat_in, *concat_zeros)
        return [
            {
                nm: np.asarray(out_arrs[i]).reshape(NCORES, *out_avals[i].shape)[c]
                for i, nm in enumerate(out_names)
            }
            for c in range(NCORES)
        ]

    _RUNNER[key] = run
    return run


_ACT_TABLES_PINNED = False


def _pin_act_tables():
    """Make the act-table placement pass see only natural_log_exp_and_others
    (which contains both Exp and Ln) so it emits ONE LoadActFuncSet instead of
    thrashing between exp_and_others and the ln set at every norm (~2.7us per
    load on HW). Other sets are masked to empty fn-sets, preserving dict order
    so act_func_set_id indices still match act_info.json."""
    global _ACT_TABLES_PINNED
    if _ACT_TABLES_PINNED:
        return
    import concourse.hw_specs as hw_specs

    orig = hw_specs.get_activation_tables

    def pinned(arch):
        tabs = orig(arch)
        if "natural_log_exp_and_others" not in tabs:
            return tabs
        return {
            name: (fns if name == "natural_log_exp_and_others" else set())
            for name, fns in tabs.items()
        }

    pinned.__wrapped__ = orig
    bacc.get_activation_tables = pinned
    _ACT_TABLES_PINNED = True


def _get_nc():
    key = _reps_key()
    if key not in _NC:
        nc = bacc.Bacc("TRN2", target_bir_lowering=False, debug=False, num_devices=1)
        xT = nc.dram_tensor("xT", [C, N], F32, kind="ExternalInput").ap()
        xTq = nc.dram_tensor("xTq", [C, QH], F32, kind="ExternalInput").ap()
        wall = nc.dram_tensor("wall", [C, 4 * C], F32, kind="ExternalInput").ap()
        pb = nc.dram_tensor("pb", [C], F32, kind="ExternalInput").ap()
        y = nc.dram_tensor("y", [QH, C], F32, kind="ExternalOutput").ap()
        with tile.TileContext(nc) as tc:
            _emit(tc, xT, xTq, wall, pb, y, *key)
        nc.finalize()
        _NC[key] = nc
    return _NC[key]


def kernel(x, qkv_w, proj_w, proj_b):
    x = np.asarray(x, dtype=np.float32)
    qkv_w = np.asarray(qkv_w, dtype=np.float32)
    proj_w = np.asarray(proj_w, dtype=np.float32)
    proj_b = np.asarray(proj_b, dtype=np.float32)

    wall = np.ascontiguousarray(
        np.stack(
            [qkv_w[C : 2 * C].T, qkv_w[0:C].T, qkv_w[2 * C : 3 * C].T, proj_w.T],
            axis=1,
        ).reshape(C, 4 * C)
    )

    in_maps = []
    for c in range(NCORES):
        b, qh = c // 2, c % 2
        xT = np.ascontiguousarray(x[b].T)
        in_maps.append(
            {
                "xT": xT,
                "xTq": np.ascontiguousarray(xT[:, qh * QH : (qh + 1) * QH]),
                "wall": wall,
                "pb": proj_b,
            }
        )
    if os.environ.get("BASS_ATTN_TIMING"):
        import time as _t

        t0 = _t.time()
        run = _get_runner()
        t1 = _t.time()
        results = run(in_maps)
        t2 = _t.time()
        out = np.empty((B, N, C), np.float32)
        for c in range(NCORES):
            b, qh = c // 2, c % 2
            out[b, qh * QH : (qh + 1) * QH] = results[c]["y"]
        t3 = _t.time()
        print(
            f"[timing] runner={t1-t0:.3f}s exec={t2-t1:.3f}s gather={t3-t2:.3f}s",
            flush=True,
        )
        return out
    results = _get_runner()(in_maps)
    out = np.empty((B, N, C), np.float32)
    for c in range(NCORES):
        b, qh = c // 2, c % 2
        out[b, qh * QH : (qh + 1) * QH] = results[c]["y"]
    return out



# revision 28
# speedup vs baseline: 1.5617x; 1.5617x over previous
"""Multi-head attention (B=4, N=2048, C=256, H=8) on 8 Trainium2 NeuronCores.

Sharding: core c handles batch b = c//2 and query-half qh = c%2 (1024 query
rows), all 8 heads. k/v are computed for the full sequence on each core (the
qkv projection is cheap); outputs concatenate with no cross-core reduction.

Device-side layout is fully "transposed" (channels on partitions):
  - x^T [C, N] feeds q^T/k^T ([d, tokens], head-major rows) and v ([tokens, d]).
  - Scores are computed as S^T [k-tokens, q-tokens] so that softmax's exp
    output E^T feeds the AV matmul directly (contraction over k on partitions).
  - Softmax denominators come for free as a 33rd "ones" column appended to v.
  - O^T [channels, q] feeds the output projection directly.
Softmax skips max-subtraction: scaled scores are ~N(0,1) (max |s| < ~10), safe
in fp32 exp. Matmuls run in bf16 (measured full PE rate on HW; fp32r 512-free
matmuls measured ~1.8x slower); PSUM accumulation stays fp32, y stays fp32.

The emission is software-pipelined around the ScalarE (exp) bottleneck
(~1035ns per [128,1024] exp on HW = the pace; everything else must hide):
  - AV matmuls trail their chunk by AV_TRAIL=2 ticks so the PE's in-order
    queue never holds an exp-dependent AV ahead of the S matmuls feeding the
    next exp.
  - k^T/q^T/vA/O^T are split into per-block tiles: the Tile framework tracks
    deps at tile granularity, so one big tile makes every S/AV matmul falsely
    wait on the latest qkv-filler DVE copy (~1.5-3us stream stall each, HW
    semaphore latency; measured justexp 267us -> 132us without fillers).
  - qkv projection spreads one item per tick (EDF worklist) with true
    per-block deps only; exp starts after a minimal prefix.
  - Per-head normalization runs as a 3-stage pipeline over following-iteration
    ticks (DVE reciprocals, then PE broadcast + scale 2 ticks later, then the
    output projection 2+ ticks after that) so no engine waits on a
    just-emitted cross-engine dependency.
  - Both AV accumulators of a head pair share one PSUM bank (partitions 0:33
    and 64:97, both legal start partitions), freeing a bank to double-buffer
    the qkv/proj scratch bank (bcp).
"""

import os
from contextlib import ExitStack

import numpy as np

import concourse.bacc as bacc
import concourse.bass as bass
import concourse.mybir as mybir
import concourse.tile as tile
from concourse.bass_utils import run_bass_kernel_spmd

B, N, C = 4, 2048, 256
H, D = 8, 32
P = 128
QH = N // 2              # query rows per core
SCALE = float(D) ** -0.5
NCORES = 8
NCH = N // P             # 16 k-chunks

F32 = mybir.dt.float32
BF16 = mybir.dt.bfloat16
EXP = mybir.ActivationFunctionType.Exp
LN = mybir.ActivationFunctionType.Ln

# Timing amplification for the local harness (repeat attention+proj body).
# REPS read per-call via _reps_key() so one process can time several variants.
# Timing-decomposition variants (break correctness, timing only):
#   "" (normal), "noact" (constant E feed), "justs" (S^T only, no exp/AV),
#   "actsb" (stage scores psum->sbuf on DVE; exp reads SBUF)
VARIANT = os.environ.get("BASS_ATTN_VARIANT", "")
# experiment knobs (timing probes)
BCPBUFS = int(os.environ.get("BASS_ATTN_BCPBUFS", "2"))
NOFILL = os.environ.get("BASS_ATTN_NOFILL", "") == "1"
# chunks whose exp runs on DVE via a one-op Schraudolph 2^x bit-trick in the
# bf16 domain: bf16bits(e^(s*SCALE)) ~= int16(s * SCHR_A + SCHR_B). Offloads
# ScalarE work; approx rel err ~3-4% per weight, ~3e-3 on the final output.
DVE_EXP_CHS = frozenset(
    int(c) for c in os.environ.get("BASS_ATTN_DVECHS", "").split(",") if c
)
import math as _math

SCHR_A = SCALE * 128.0 / _math.log(2.0)
SCHR_B = 127.0 * 128.0 - 7.42
I16 = mybir.dt.int16


def _emit(tc, xT, xTq, wall, pb, y, REPS, REPS_MODE):
    nc = tc.nc
    with ExitStack() as ctx:
        singles = ctx.enter_context(tc.tile_pool(name="singles", bufs=1))
        epool = ctx.enter_context(
            tc.tile_pool(name="epool", bufs=2 if VARIANT == "actsb" else 4)
        )
        odp = ctx.enter_context(tc.tile_pool(name="odp", bufs=3))
        small = ctx.enter_context(tc.tile_pool(name="small", bufs=4))
        ypool = ctx.enter_context(tc.tile_pool(name="ypool", bufs=3))
        # PSUM budget (8 banks): s-tiles 2x2 + po 2x1 (paired accums) + bcp 2x1
        ps = ctx.enter_context(tc.tile_pool(name="ps", bufs=2, space="PSUM"))
        po = ctx.enter_context(tc.tile_pool(name="po", bufs=2, space="PSUM"))
        bcp = ctx.enter_context(tc.tile_pool(name="bcp", bufs=BCPBUFS, space="PSUM"))

        # ---- input loads + fp32r rounding (staged) -----------------------
        def load_r(name, dram_ap, cshape):
            ld = singles.tile(cshape, F32, tag=name + "_ld", name=name + "_ld")
            nc.sync.dma_start(ld[:], dram_ap)
            rt = singles.tile(cshape, BF16, tag=name, name=name)
            nc.vector.tensor_copy(rt[:], ld[:])
            return rt

        # all four weight matrices arrive packed in one dram tensor (one DMA,
        # one rounding copy); order: wk, wq, wv, pw
        wall_ld = singles.tile([P, 2, 4 * C], F32, tag="wall_ld", name="wall_ld")
        wall_sb = singles.tile([P, 2, 4 * C], BF16, tag="wall", name="wall_sb")
        wall4 = wall_sb[:].rearrange("p c (w n) -> p c w n", n=C)
        wk_sb = wall4[:, :, 0]
        wq_sb = wall4[:, :, 1]
        wv_sb = wall4[:, :, 2]
        pw_sb = wall4[:, :, 3]
        # x^T loads staged + rounded in 512-column blocks so the first qkv
        # matmuls (and the attention stream behind them) start early.
        xT_ld = singles.tile([P, 2, N], F32, tag="xT_ld", name="xT_ld")
        xT_sb = singles.tile([P, 2, N], BF16, tag="xT", name="xT_sb")
        xT_r = xT.rearrange("(c p) n -> p c n", p=P)

        def load_x_block(nb):
            sl = (slice(None), slice(None), slice(512 * nb, 512 * nb + 512))
            nc.sync.dma_start(xT_ld[sl], xT_r[sl])
            nc.vector.tensor_copy(xT_sb[sl], xT_ld[sl])

        load_x_block(0)
        nc.sync.dma_start(wall_ld[:], wall.rearrange("(c p) n -> p c n", p=P))
        # round the k-projection slice first: it gates the first matmuls
        nc.vector.tensor_copy(wall_sb[:, :, 0:C], wall_ld[:, :, 0:C])
        nc.vector.tensor_copy(wall_sb[:, :, C : 4 * C], wall_ld[:, :, C : 4 * C])
        xTq_ld = singles.tile([P, 2, QH], F32, tag="xTq_ld", name="xTq_ld")
        xTq_sb = singles.tile([P, 2, QH], BF16, tag="xTq", name="xTq_sb")
        xTq_r = xTq.rearrange("(c p) n -> p c n", p=P)

        def load_xq_block(nb):
            sl = (slice(None), slice(None), slice(512 * nb, 512 * nb + 512))
            nc.sync.dma_start(xTq_ld[sl], xTq_r[sl])
            nc.vector.tensor_copy(xTq_sb[sl], xTq_ld[sl])

        load_xq_block(0)
        for nb in range(1, 4):
            load_x_block(nb)
        load_xq_block(1)
        pb_sb = singles.tile([P, C], F32, tag="pb")
        nc.sync.dma_start(
            pb_sb[:],
            bass.AP(tensor=pb.tensor, offset=pb.offset, ap=[[0, P]] + list(pb.ap)),
        )

        # ---- qkv projection emitters ------------------------------------
        # q^T/k^T stacks: chunk cc holds heads 4cc..4cc+3 at rows 32*(h%4).
        # Split into per-512-token-block tiles (and vA per 128-token chunk):
        # the Tile framework tracks dependencies at tile granularity, so a
        # single big tile makes every S/AV matmul falsely wait on the latest
        # qkv-filler DVE copy (~1.5-3us stream stall each on HW).
        qTb = [
            singles.tile([P, 2, 512], BF16, tag=f"qT{nb}", name=f"qT{nb}")
            for nb in range(2)
        ]
        kTb = [
            singles.tile([P, 2, 512], BF16, tag=f"kT{nb}", name=f"kT{nb}")
            for nb in range(4)
        ]
        # v_aug: [token-chunk tile, head-major (v_h | 1)] for AV + denominator.
        vAb = [
            singles.tile([P, H * (D + 1)], BF16, tag=f"vA{tt}", name=f"vA{tt}")
            for tt in range(NCH)
        ]
        onesF = singles.tile([P, H], F32, tag="onesF")
        nc.vector.memset(onesF[:], 1.0)
        for tt in range(NCH):
            nc.vector.tensor_copy(
                vAb[tt][:].rearrange("p (h a) -> p h a", a=D + 1)[:, :, D],
                onesF[:],
            )

        def emit_kqT(w_sb, x_sb, out_tiles, cc, nb):
            pk = bcp.tile([P, 512], F32, tag="bc", name="pk")
            for ci in range(2):
                nc.tensor.matmul(
                    pk[:],
                    lhsT=w_sb[:, ci, 128 * cc : 128 * cc + 128],
                    rhs=x_sb[:, ci, 512 * nb : 512 * nb + 512],
                    start=(ci == 0),
                    stop=(ci == 1),
                )
            nc.vector.tensor_copy(out_tiles[nb][:, cc, :], pk[:])

        def emit_v(tt):
            pv = bcp.tile([P, 512], F32, tag="bc", name="pv")
            for ci in range(2):
                nc.tensor.matmul(
                    pv[:, 0:256],
                    lhsT=xT_sb[:, ci, 128 * tt : 128 * tt + 128],
                    rhs=wv_sb[:, ci, :],
                    start=(ci == 0),
                    stop=(ci == 1),
                )
            nc.vector.tensor_copy(
                vAb[tt][:].rearrange("p (h a) -> p h a", a=D + 1)[:, :, 0:D],
                pv[:, 0:256].rearrange("p (h d) -> p h d", d=D),
            )

        # ---- attention helpers ------------------------------------------
        ones_f2 = singles.tile([1, 32], F32, tag="onesf2")
        nc.vector.memset(ones_f2[:], 1.0)
        ones_sb = singles.tile([1, 32], BF16, tag="ones")
        nc.vector.tensor_copy(ones_sb[:], ones_f2[:])
        et_const = None
        if VARIANT in ("noact", "justs"):
            etc_f = singles.tile([P, 1024], F32, tag="etcf", name="etc_f")
            nc.vector.memset(etc_f[:], 0.5)
            et_const = singles.tile([P, 1024], BF16, tag="etc", name="et_const")
            nc.vector.tensor_copy(et_const[:], etc_f[:])
        # O^T split per q-block so proj(qb) only depends on qb's own norms
        OTb = [
            singles.tile([P, 2, 512], BF16, tag=f"OT{qb}", name=f"OT{qb}")
            for qb in range(2)
        ]

        def emit_proj_qt(qb, qt):
            tq = 4 * qb + qt
            py = bcp.tile([P, 512], F32, tag="bc", name="py")
            for ci in range(2):
                nc.tensor.matmul(
                    py[:, 0:256],
                    lhsT=OTb[qb][:, ci, 128 * qt : 128 * qt + 128],
                    rhs=pw_sb[:, ci, :],
                    start=(ci == 0),
                    stop=(ci == 1),
                )
            ysb = ypool.tile([P, C], F32, tag="y", name="ysb")
            nc.vector.tensor_add(ysb[:], py[:, 0:256], pb_sb[:])
            nc.sync.dma_start(y[128 * tq : 128 * tq + 128, :], ysb[:])

        def emit_norm_rcp(pots):
            # stage 1 (DVE only): reciprocals of both denominators -> bf16
            rcps = []
            for e in range(2):
                rcpf = small.tile([1, 512], F32, tag="rcpf", name="rcpf")
                nc.vector.reciprocal(rcpf[:], pots[e][D : D + 1, :])
                rcp = small.tile([1, 512], BF16, tag="rcp", name="rcp")
                nc.vector.tensor_copy(rcp[:], rcpf[:])
                rcps.append(rcp)
            return rcps

        def emit_norm_head(pot, rcp, hp, qb, e):
            # stage 2: broadcast 1/den (PE) and scale O^T of head 2hp+e
            h = 2 * hp + e
            r, cc = 32 * (h % 4), h // 4
            bc = bcp.tile([32, 512], F32, tag="bc", name="bc")
            nc.tensor.matmul(  # broadcast 1/den to 32 rows
                bc[:], lhsT=ones_sb[:], rhs=rcp[:], start=True, stop=True
            )
            onr = small.tile([32, 512], F32, tag="onr", name="onr")
            nc.vector.tensor_copy(onr[:], pot[0:D, :])
            nc.vector.tensor_mul(
                OTb[qb][r : r + 32, cc, :],
                onr[:],
                bc[:],
            )

        def emit_av(pots, hp, et, ch):
            # et: [P, 1024] AP (heads side by side) for chunk ch
            for e in range(2):
                h = 2 * hp + e
                nc.tensor.matmul(
                    pots[e],
                    lhsT=vAb[ch][:, (D + 1) * h : (D + 1) * (h + 1)],
                    rhs=et[:, 512 * e : 512 * e + 512],
                    start=(ch == 0),
                    stop=(ch == NCH - 1),
                    skip_group_check=True,
                )

        def emit_body():
            # ---- prefix: just enough qkv for iteration 0's first chunks ------
            emit_kqT(wk_sb, xT_sb, kTb, 0, 0)
            emit_kqT(wq_sb, xTq_sb, qTb, 0, 0)
            for tt in range(4):
                emit_v(tt)
            # remaining qkv work, spread one item per chunk tick (EDF order);
            # deadlines with AV trailing 2 ticks: v_tt before AV(tt) is emitted
            # at tick tt+2, kT(0,nb) before tick 4nb, cc=1 before tick 32.
            worklist = [
                lambda: emit_kqT(wk_sb, xT_sb, kTb, 0, 1),
                lambda: emit_v(4),
                lambda: emit_v(5),
                lambda: emit_v(6),
                lambda: emit_v(7),
                lambda: emit_kqT(wk_sb, xT_sb, kTb, 0, 2),
                lambda: emit_v(8),
                lambda: emit_v(9),
                lambda: emit_v(10),
                lambda: emit_v(11),
                lambda: emit_kqT(wk_sb, xT_sb, kTb, 0, 3),
                lambda: emit_v(12),
                lambda: emit_v(13),
                lambda: emit_v(14),
                lambda: emit_v(15),
                lambda: emit_kqT(wk_sb, xT_sb, kTb, 1, 0),
                lambda: emit_kqT(wk_sb, xT_sb, kTb, 1, 1),
                lambda: emit_kqT(wk_sb, xT_sb, kTb, 1, 2),
                lambda: emit_kqT(wk_sb, xT_sb, kTb, 1, 3),
                lambda: emit_kqT(wq_sb, xTq_sb, qTb, 1, 0),
                lambda: emit_kqT(wq_sb, xTq_sb, qTb, 0, 1),
                lambda: emit_kqT(wq_sb, xTq_sb, qTb, 1, 1),
            ]
            if NOFILL:
                for w in worklist:
                    w()
                worklist = []

            # ---- attention main loop -----------------------------------------
            # AV trails its chunk by AV_TRAIL ticks so the PE's in-order queue
            # never holds an exp-dependent AV ahead of the S matmuls that feed
            # the next exp: ACT streams back-to-back and paces the kernel.
            AV_TRAIL = 2
            pend_av = []        # queue of (pots, hp, et, ch)
            deferred = []       # norm/proj actions, one per tick (ticks >= 2,
                                # after the cross-boundary trailing AVs)
            body_reps = REPS if (REPS > 1 and REPS_MODE == "unroll") else 1
            its = [
                (qb, hp)
                for _ in range(body_reps)
                for qb in range(QH // 512)
                for hp in range(H // 2)
            ]
            for qb, hp in its:
                pots = None
                for ch in range(NCH):
                    s = ps.tile([P, 1024], F32, tag="s", name="s")
                    for e in range(2):
                        h = 2 * hp + e
                        r, cc = 32 * (h % 4), h // 4
                        co = 128 * (ch % 4)
                        nc.tensor.matmul(
                            s[:, 512 * e : 512 * e + 512],
                            lhsT=kTb[ch // 4][r : r + 32, cc, co : co + 128],
                            rhs=qTb[qb][r : r + 32, cc, :],
                            start=True,
                            stop=True,
                            tile_position=(r, 0),
                        )
                    if VARIANT == "noact":
                        et = et_const
                    elif VARIANT == "justs":
                        # keep S^T alive with a tiny DVE read; no exp, no AV
                        et = et_const
                        scr = small.tile([1, 32], F32, tag="scr", name="scr")
                        nc.vector.tensor_copy(scr[:], s[0:1, 0:32])
                    elif VARIANT == "justexp":
                        # S + exp stream, no AV/norm/proj: measures ACT pace
                        etw = epool.tile([P, 1024], BF16, tag="E", name="etw")
                        nc.scalar.activation(etw[:], s[:], EXP, scale=SCALE)
                        et = etw[:]
                    elif VARIANT == "actsb":
                        # old design (baseline A/B): stage scores to SBUF on
                        # DVE, batched exp over 4 chunks reads SBUF
                        if ch % 4 == 0:
                            scw = epool.tile([P, 4, 1024], F32, tag="SC", name="scw")
                        nc.vector.tensor_copy(scw[:, ch % 4, :], s[:])
                        if ch % 4 == 3:
                            etw4 = epool.tile([P, 4, 1024], BF16, tag="E4", name="etw4")
                            nc.scalar.activation(etw4[:], scw[:], EXP, scale=SCALE)
                            for j in range(4):
                                emit_av(pots, hp, etw4[:, j, :], ch - 3 + j)
                    elif ch in DVE_EXP_CHS:
                        # Schraudolph exp on DVE (int16 bits of bf16 2^x)
                        eiw = epool.tile([P, 1024], I16, tag="E", name="eiw")
                        nc.vector.tensor_scalar(
                            eiw[:],
                            s[:],
                            SCHR_A,
                            SCHR_B,
                            mybir.AluOpType.mult,
                            mybir.AluOpType.add,
                        )
                        et = eiw[:].bitcast(BF16)
                    else:
                        # exp reads scores directly from PSUM (ScalarE is
                        # closer to PSUM; avoids a DVE staging copy, which
                        # made DVE the bottleneck engine at ~211us busy)
                        etw = epool.tile([P, 1024], BF16, tag="E", name="etw")
                        nc.scalar.activation(etw[:], s[:], EXP, scale=SCALE)
                        et = etw[:]
                    if pots is None and VARIANT not in ("justs", "justexp"):
                        # both heads' accumulators share one PSUM bank at
                        # legal start partitions 0 and 64
                        potpair = po.tile([P, 512], F32, tag="o", name="potp")
                        pots = (
                            potpair[0 : D + 1, :],
                            potpair[64 : 64 + D + 1, :],
                        )
                    if VARIANT not in ("justs", "actsb", "justexp"):
                        if len(pend_av) == AV_TRAIL:
                            emit_av(*pend_av.pop(0))
                        pend_av.append((pots, hp, et, ch))
                    if deferred and ch >= 2:
                        d = deferred.pop(0)
                        if d is not None:
                            d()
                    elif worklist:
                        worklist.pop(0)()
                if VARIANT in ("justs", "nonorm", "justexp"):
                    continue
                if (qb, hp) != its[-1]:
                    # norm pipeline with slack between the DVE reciprocal
                    # stage, the PE broadcast stage, and the proj consumers,
                    # so no engine waits on a just-emitted cross-engine dep
                    holder = []
                    deferred.append(
                        lambda h_=holder, p_=pots: h_.extend(emit_norm_rcp(p_))
                    )
                    deferred.append(None)
                    deferred.append(
                        lambda h_=holder, p_=pots, b=hp, c=qb: emit_norm_head(
                            p_[0], h_[0], b, c, 0
                        )
                    )
                    deferred.append(
                        lambda h_=holder, p_=pots, b=hp, c=qb: emit_norm_head(
                            p_[1], h_[1], b, c, 1
                        )
                    )
                    if hp == H // 2 - 1:
                        deferred.append(None)
                        for qt in range(4):
                            deferred.append(lambda a=qb, b=qt: emit_proj_qt(a, b))
            if VARIANT in ("justs", "nonorm", "justexp"):
                if VARIANT == "nonorm":
                    for av in pend_av:
                        emit_av(*av)
                for tq in range(8):
                    ysb = ypool.tile([P, C], F32, tag="y", name="ysbj")
                    nc.vector.memset(ysb[:], 0.0)
                    nc.sync.dma_start(y[128 * tq : 128 * tq + 128, :], ysb[:])
                return
            for av in pend_av:
                emit_av(*av)
            pend_av = []
            for act in deferred:
                if act is not None:
                    act()
            # fast epilogue for the final head pair: interleave both heads'
            # chains and broadcast with plain-fp32 matmuls (PE is idle here),
            # skipping the fp32r rounding copy on the reciprocal.
            l_qb, l_hp = its[-1]
            rcpfs = []
            for e in range(2):
                rcpf = small.tile([1, 512], F32, tag="rcpf", name="rcpf")
                nc.vector.reciprocal(rcpf[:], pots[e][D : D + 1, :])
                rcpfs.append(rcpf)
            for e in range(2):
                bc = ps.tile([P, 1024], F32, tag="s", name="bcT")
                nc.tensor.matmul(
                    bc[0:32, 0:512], lhsT=ones_f2[:], rhs=rcpfs[e][:],
                    start=True, stop=True,
                )
                onr = small.tile([32, 512], F32, tag="onr", name="onr")
                nc.vector.tensor_copy(onr[:], pots[e][0:D, :])
                h = 2 * l_hp + e
                r, cc = 32 * (h % 4), h // 4
                nc.vector.tensor_mul(
                    OTb[l_qb][r : r + 32, cc, :],
                    onr[:],
                    bc[0:32, 0:512],
                )
            for qt in range(4):
                tq = 4 * l_qb + qt
                py = ps.tile([P, 1024], F32, tag="s", name="pyT")
                for ci in range(2):
                    nc.tensor.matmul(
                        py[:, 0:256],
                        lhsT=OTb[l_qb][:, ci, 128 * qt : 128 * qt + 128],
                        rhs=pw_sb[:, ci, :],
                        start=(ci == 0),
                        stop=(ci == 1),
                    )
                ysb = ypool.tile([P, C], F32, tag="y", name="ysb")
                nc.vector.tensor_add(ysb[:], py[:, 0:256], pb_sb[:])
                nc.sync.dma_start(y[128 * tq : 128 * tq + 128, :], ysb[:])

        if REPS == 1 or REPS_MODE == "unroll":
            emit_body()
        else:
            with tc.For_i(0, REPS, 1):
                emit_body()


_NC = {}
_RUNNER = {}


def _reps_key():
    reps = int(os.environ.get("BASS_ATTN_REPS", "1"))
    mode = os.environ.get("BASS_ATTN_REPS_MODE", "loop")
    return (reps, mode if reps > 1 else "loop")


def _get_runner():
    """Cached SPMD runner: builds the jitted shard_map executable once so warm
    kernel() calls skip JAX retracing/compilation (run_bass_kernel_spmd builds
    a fresh closure per call, which always misses the jit cache)."""
    key = _reps_key()
    if key in _RUNNER:
        return _RUNNER[key]
    import jax
    from jax.sharding import Mesh, PartitionSpec
    from jax.experimental.shard_map import shard_map
    from concourse import bass2jax, mybir as _mb

    nc = _get_nc()
    bass2jax.install_neuronx_cc_hook()

    assert nc.dbg_addr is None
    partition_name = nc.partition_id_tensor.name if nc.partition_id_tensor else None
    in_names, out_names, out_avals = [], [], []
    for alloc in nc.m.functions[0].allocations:
        if not isinstance(alloc, _mb.MemoryLocationSet):
            continue
        name = alloc.memorylocations[0].name
        if alloc.kind == "ExternalInput":
            if name != partition_name:
                in_names.append(name)
        elif alloc.kind == "ExternalOutput":
            out_names.append(name)
            out_avals.append(
                jax.core.ShapedArray(tuple(alloc.tensor_shape), _mb.dt.np(alloc.dtype))
            )
    n_params = len(in_names)
    n_outs = len(out_avals)
    all_names = in_names + out_names
    if partition_name is not None:
        all_names = all_names + [partition_name]

    def _body(*args):
        operands = list(args)
        if partition_name is not None:
            operands.append(bass2jax.partition_id_tensor())
        outs = bass2jax._bass_exec_p.bind(
            *operands,
            out_avals=tuple(out_avals),
            in_names=tuple(all_names),
            out_names=tuple(out_names),
            lowering_input_output_aliases=(),
            sim_require_finite=True,
            sim_require_nnan=True,
            nc=nc,
        )
        return tuple(outs)

    devices = jax.devices()[:NCORES]
    mesh = Mesh(np.asarray(devices), ("core",))
    sharded = jax.jit(
        shard_map(
            _body,
            mesh=mesh,
            in_specs=(PartitionSpec("core"),) * (n_params + n_outs),
            out_specs=(PartitionSpec("core"),) * n_outs,
            check_rep=False,
        ),
        donate_argnums=tuple(range(n_params, n_params + n_outs)),
        keep_unused=True,
    )

    def run(in_maps):
        concat_in = [
            np.concatenate([np.asarray(m[nm]) for m in in_maps], axis=0)
            for nm in in_names
        ]
        concat_zeros = [
            np.zeros((NCORES * a.shape[0], *a.shape[1:]), a.dtype) for a in out_avals
        ]
        out_arrs = sharded(*concat_in, *concat_zeros)
        return [
            {
                nm: np.asarray(out_arrs[i]).reshape(NCORES, *out_avals[i].shape)[c]
                for i, nm in enumerate(out_names)
            }
            for c in range(NCORES)
        ]

    _RUNNER[key] = run
    return run


_ACT_TABLES_PINNED = False


def _pin_act_tables():
    """Make the act-table placement pass see only natural_log_exp_and_others
    (which contains both Exp and Ln) so it emits ONE LoadActFuncSet instead of
    thrashing between exp_and_others and the ln set at every norm (~2.7us per
    load on HW). Other sets are masked to empty fn-sets, preserving dict order
    so act_func_set_id indices still match act_info.json."""
    global _ACT_TABLES_PINNED
    if _ACT_TABLES_PINNED:
        return
    import concourse.hw_specs as hw_specs

    orig = hw_specs.get_activation_tables

    def pinned(arch):
        tabs = orig(arch)
        if "natural_log_exp_and_others" not in tabs:
            return tabs
        return {
            name: (fns if name == "natural_log_exp_and_others" else set())
            for name, fns in tabs.items()
        }

    pinned.__wrapped__ = orig
    bacc.get_activation_tables = pinned
    _ACT_TABLES_PINNED = True


def _get_nc():
    key = _reps_key()
    if key not in _NC:
        nc = bacc.Bacc("TRN2", target_bir_lowering=False, debug=False, num_devices=1)
        xT = nc.dram_tensor("xT", [C, N], F32, kind="ExternalInput").ap()
        xTq = nc.dram_tensor("xTq", [C, QH], F32, kind="ExternalInput").ap()
        wall = nc.dram_tensor("wall", [C, 4 * C], F32, kind="ExternalInput").ap()
        pb = nc.dram_tensor("pb", [C], F32, kind="ExternalInput").ap()
        y = nc.dram_tensor("y", [QH, C], F32, kind="ExternalOutput").ap()
        with tile.TileContext(nc) as tc:
            _emit(tc, xT, xTq, wall, pb, y, *key)
        nc.finalize()
        _NC[key] = nc
    return _NC[key]


def kernel(x, qkv_w, proj_w, proj_b):
    x = np.asarray(x, dtype=np.float32)
    qkv_w = np.asarray(qkv_w, dtype=np.float32)
    proj_w = np.asarray(proj_w, dtype=np.float32)
    proj_b = np.asarray(proj_b, dtype=np.float32)

    wall = np.ascontiguousarray(
        np.stack(
            [qkv_w[C : 2 * C].T, qkv_w[0:C].T, qkv_w[2 * C : 3 * C].T, proj_w.T],
            axis=1,
        ).reshape(C, 4 * C)
    )

    in_maps = []
    for c in range(NCORES):
        b, qh = c // 2, c % 2
        xT = np.ascontiguousarray(x[b].T)
        in_maps.append(
            {
                "xT": xT,
                "xTq": np.ascontiguousarray(xT[:, qh * QH : (qh + 1) * QH]),
                "wall": wall,
                "pb": proj_b,
            }
        )
    if os.environ.get("BASS_ATTN_TIMING"):
        import time as _t

        t0 = _t.time()
        run = _get_runner()
        t1 = _t.time()
        results = run(in_maps)
        t2 = _t.time()
        out = np.empty((B, N, C), np.float32)
        for c in range(NCORES):
            b, qh = c // 2, c % 2
            out[b, qh * QH : (qh + 1) * QH] = results[c]["y"]
        t3 = _t.time()
        print(
            f"[timing] runner={t1-t0:.3f}s exec={t2-t1:.3f}s gather={t3-t2:.3f}s",
            flush=True,
        )
        return out
    results = _get_runner()(in_maps)
    out = np.empty((B, N, C), np.float32)
    for c in range(NCORES):
        b, qh = c // 2, c % 2
        out[b, qh * QH : (qh + 1) * QH] = results[c]["y"]
    return out

